# revision 3
# baseline (speedup 1.0000x reference)
"""Block-diagonal matmul kernel for Trainium2 (8 NeuronCores, SPMD).

Reference computation: out = x @ (blocks * mask) with
  x      [64, 8192]  f32
  blocks [8192, 8192] f32
  mask   [8192, 8192] bool, block-diagonal (32 blocks of 256x256)

Only the 32 diagonal 256x256 blocks of `blocks` survive the mask, so the
real work is 32 independent [64,256] @ [256,256] matmuls.  Sharding
(per the expert/tensor-parallel hint): core d owns blocks 4d..4d+3 and
produces out[:, d*1024:(d+1)*1024].  x is sliced per-core (each block
only reads the matching 256 columns of x), outputs are concatenated on
the host - no cross-device communication needed.

Device-side layout (host prepares everything so every DMA is a plain
contiguous copy):
  xT  [128, 512]  - 8 K-chunks of x-slice^T side by side ([128,64] each)
  bks [128, 2048] - 8 K-chunks of the 4 blocks side by side ([128,256] each)
  y   [64, 1024]  - output slice
Chunk c = 2b+k is K-half k of block b; matmul accumulates the 2 halves
of each block into one PSUM tile: y_b = xT_c.T @ B_c summed over k.
"""

import numpy as np

N_BLOCKS = 32
BLOCK = 256
N = N_BLOCKS * BLOCK  # 8192
BATCH = 64
N_CORES = 8
BPC = N_BLOCKS // N_CORES  # blocks per core = 4
COLS = BPC * BLOCK  # output columns per core = 1024
KCH = BLOCK // 128  # K-chunks per block = 2
NCH = BPC * KCH  # chunks per core = 8

_cached_nc = None


def _ensure_axon_ntff_hook():
    """The image's `antenv` package lacks `axon_hooks`, which
    run_bass_kernel_spmd imports unconditionally when tracing under axon.
    Inject a minimal shim and register the ctypes-based NTFF hook."""
    import sys
    import types

    try:
        import antenv.axon_hooks  # noqa: F401

        return
    except ImportError:
        pass
    try:
        import antenv
    except ImportError:
        return
    mod = types.ModuleType("antenv.axon_hooks")
    holder = {"h": None}
    mod.set_axon_ntff_profile_hook = lambda h: holder.__setitem__("h", h)
    mod.get_axon_ntff_profile_hook = lambda: holder["h"]
    sys.modules["antenv.axon_hooks"] = mod
    antenv.axon_hooks = mod
    try:
        from trn_agent_boot.trn_boot import _ntff_profile_via_ctypes

        h = _ntff_profile_via_ctypes("/opt/axon/libaxon_pjrt.so")
        if h is not None:
            mod.set_axon_ntff_profile_hook(h)
    except Exception:
        pass


def _build_nc():
    global _cached_nc
    if _cached_nc is not None:
        return _cached_nc

    import concourse.bacc as bacc
    import concourse.mybir as mybir
    import concourse.tile as tile
    import concourse.bass as bass

    f32 = mybir.dt.float32
    nc = bacc.Bacc("TRN2", debug=False, num_devices=N_CORES)

    xT = nc.dram_tensor("xt", [128, NCH * BATCH], f32, kind="ExternalInput")
    bks = nc.dram_tensor("bks", [128, NCH * BLOCK], f32, kind="ExternalInput")
    y = nc.dram_tensor("y", [BATCH, COLS], f32, kind="ExternalOutput")

    with tile.TileContext(nc) as tc:
        with (
            tc.tile_pool(name="sb", bufs=1) as pool,
            tc.tile_pool(name="ps", bufs=BPC, space=bass.MemorySpace.PSUM) as pp,
        ):
            xt = pool.tile([128, NCH * BATCH], f32)
            nc.sync.dma_start(xt[:], xT.ap())
            out_sb = pool.tile([BATCH, COLS], f32)

            bt = []
            for b in range(BPC):
                # one DMA per block (256 KB) so matmuls can start while
                # later blocks are still in flight
                t = pool.tile([128, KCH * BLOCK], f32, name=f"bt{b}", bufs=1)
                nc.sync.dma_start(
                    t[:], bks.ap()[:, b * KCH * BLOCK : (b + 1) * KCH * BLOCK]
                )
                bt.append(t)

            for b in range(BPC):
                acc = pp.tile([BATCH, BLOCK], f32)
                for k in range(KCH):
                    c = b * KCH + k
                    nc.tensor.matmul(
                        acc[:],
                        xt[:, c * BATCH : (c + 1) * BATCH],
                        bt[b][:, k * BLOCK : (k + 1) * BLOCK],
                        start=(k == 0),
                        stop=(k == KCH - 1),
                    )
                nc.vector.tensor_copy(
                    out_sb[:, b * BLOCK : (b + 1) * BLOCK], acc[:]
                )

            nc.sync.dma_start(y.ap(), out_sb[:])

    nc.compile()
    _cached_nc = nc
    return nc


def _prep_in_maps(x, blocks, mask):
    x = np.ascontiguousarray(x, dtype=np.float32)
    in_maps = []
    for d in range(N_CORES):
        s0 = d * COLS
        # x slice transposed: [1024, 64] -> 8 chunks of [128, 64] -> [128, 512]
        xs = x[:, s0 : s0 + COLS].T.reshape(NCH, 128, BATCH)
        xt = np.ascontiguousarray(xs.transpose(1, 0, 2)).reshape(128, NCH * BATCH)
        # diagonal blocks (mask applied), K-chunked: [128, 8, 256] -> [128, 2048]
        bk = np.empty((128, NCH, BLOCK), dtype=np.float32)
        for b in range(BPC):
            s = s0 + b * BLOCK
            blk = blocks[s : s + BLOCK, s : s + BLOCK] * mask[s : s + BLOCK, s : s + BLOCK]
            for k in range(KCH):
                bk[:, b * KCH + k, :] = blk[k * 128 : (k + 1) * 128, :]
        in_maps.append(
            {
                "xt": xt,
                "bks": np.ascontiguousarray(bk).reshape(128, NCH * BLOCK),
            }
        )
    return in_maps


def _run(x, blocks, mask, trace=False):
    from concourse import bass_utils

    _ensure_axon_ntff_hook()
    nc = _build_nc()
    in_maps = _prep_in_maps(x, blocks, mask)
    res = bass_utils.run_bass_kernel_spmd(
        nc, in_maps, core_ids=list(range(N_CORES)), trace=trace
    )
    out = np.empty((BATCH, N), dtype=np.float32)
    for d in range(N_CORES):
        out[:, d * COLS : (d + 1) * COLS] = res.results[d]["y"]
    return out, res


def kernel(x, blocks, mask):
    out, _ = _run(x, blocks, mask, trace=False)
    return out


# revision 4
# speedup vs baseline: 1.3029x; 1.3029x over previous
"""Block-diagonal matmul kernel for Trainium2 (8 NeuronCores, SPMD).

Reference computation: out = x @ (blocks * mask) with
  x      [64, 8192]  f32
  blocks [8192, 8192] f32
  mask   [8192, 8192] bool, block-diagonal (32 blocks of 256x256)

Only the 32 diagonal 256x256 blocks of `blocks` survive the mask, so the
real work is 32 independent [64,256] @ [256,256] matmuls.  Sharding
(per the expert/tensor-parallel hint): core d owns blocks 4d..4d+3 and
produces out[:, d*1024:(d+1)*1024].  x is sliced per-core (each block
only reads the matching 256 columns of x), outputs are concatenated on
the host - no cross-device communication needed.

Device-side layout (host prepares everything so every DMA is a plain
contiguous copy; inputs are pre-converted to bf16 on the host, which
halves HBM traffic and gives single-pass matmuls; accumulation stays
fp32 in PSUM):
  in0 [128, 1536] bf16 - x-slice^T (8 chunks of [128,64]) + blocks 0,1
  in1 [128, 1024] bf16 - blocks 2,3 (each block = 2 K-chunks of [128,256])
  y   [64, 1024]  f32  - output slice
Chunk c = 2b+k is K-half k of block b; matmul accumulates the 2 halves
of each block into one PSUM tile: y_b = sum_k xT_c.T @ B_c.
"""

import numpy as np

N_BLOCKS = 32
BLOCK = 256
N = N_BLOCKS * BLOCK  # 8192
BATCH = 64
N_CORES = 8
BPC = N_BLOCKS // N_CORES  # blocks per core = 4
COLS = BPC * BLOCK  # output columns per core = 1024
KCH = BLOCK // 128  # K-chunks per block = 2
NCH = BPC * KCH  # chunks per core = 8
XT_COLS = NCH * BATCH  # 512

_cached_nc = None


def _ensure_axon_ntff_hook():
    """The image's `antenv` package lacks `axon_hooks`, which
    run_bass_kernel_spmd imports unconditionally when tracing under axon.
    Inject a minimal shim and register the ctypes-based NTFF hook."""
    import sys
    import types

    try:
        import antenv.axon_hooks  # noqa: F401

        return
    except ImportError:
        pass
    try:
        import antenv
    except ImportError:
        return
    mod = types.ModuleType("antenv.axon_hooks")
    holder = {"h": None}
    mod.set_axon_ntff_profile_hook = lambda h: holder.__setitem__("h", h)
    mod.get_axon_ntff_profile_hook = lambda: holder["h"]
    sys.modules["antenv.axon_hooks"] = mod
    antenv.axon_hooks = mod
    try:
        from trn_agent_boot.trn_boot import _ntff_profile_via_ctypes

        h = _ntff_profile_via_ctypes("/opt/axon/libaxon_pjrt.so")
        if h is not None:
            mod.set_axon_ntff_profile_hook(h)
    except Exception:
        pass


def _build_nc():
    global _cached_nc
    if _cached_nc is not None:
        return _cached_nc

    import concourse.bacc as bacc
    import concourse.mybir as mybir
    import concourse.tile as tile
    import concourse.bass as bass

    f32 = mybir.dt.float32
    bf16 = mybir.dt.bfloat16
    nc = bacc.Bacc("TRN2", debug=False, num_devices=N_CORES)

    # in0 = xT (512 cols) + blocks 0,1 (1024 cols); in1 = blocks 2,3
    in0 = nc.dram_tensor("in0", [128, XT_COLS + 2 * KCH * BLOCK], bf16,
                         kind="ExternalInput")
    in1 = nc.dram_tensor("in1", [128, 2 * KCH * BLOCK], bf16,
                         kind="ExternalInput")
    y = nc.dram_tensor("y", [BATCH, COLS], f32, kind="ExternalOutput")

    with tile.TileContext(nc) as tc:
        with (
            tc.tile_pool(name="sb", bufs=1) as pool,
            tc.tile_pool(name="ps", bufs=BPC, space=bass.MemorySpace.PSUM) as pp,
        ):
            t0 = pool.tile([128, XT_COLS + 2 * KCH * BLOCK], bf16)
            t1 = pool.tile([128, 2 * KCH * BLOCK], bf16)
            nc.sync.dma_start(t0[:], in0.ap())
            nc.sync.dma_start(t1[:], in1.ap())
            out_sb = pool.tile([BATCH, COLS], f32)

            xt = t0[:, 0:XT_COLS]

            def b_chunk(b, k):
                # [128, 256] slice for K-half k of block b
                c = (b % 2) * KCH + k
                t = t0 if b < 2 else t1
                off = (XT_COLS if b < 2 else 0) + c * BLOCK
                return t[:, off : off + BLOCK]

            for b in range(BPC):
                acc = pp.tile([BATCH, BLOCK], f32)
                for k in range(KCH):
                    c = b * KCH + k
                    nc.tensor.matmul(
                        acc[:],
                        xt[:, c * BATCH : (c + 1) * BATCH],
                        b_chunk(b, k),
                        start=(k == 0),
                        stop=(k == KCH - 1),
                    )
                nc.vector.tensor_copy(
                    out_sb[:, b * BLOCK : (b + 1) * BLOCK], acc[:]
                )

            nc.scalar.dma_start(y.ap(), out_sb[:])

    nc.compile()
    _cached_nc = nc
    return nc


def _prep_in_maps(x, blocks, mask):
    import ml_dtypes

    bf16 = ml_dtypes.bfloat16
    x = np.ascontiguousarray(x, dtype=np.float32)
    in_maps = []
    for d in range(N_CORES):
        s0 = d * COLS
        # x slice transposed: [1024, 64] -> 8 chunks of [128, 64] -> [128, 512]
        xs = x[:, s0 : s0 + COLS].T.reshape(NCH, 128, BATCH)
        xt = np.ascontiguousarray(xs.transpose(1, 0, 2)).reshape(128, XT_COLS)
        # diagonal blocks (mask applied), K-chunked to [128, 256] slabs
        bk = np.empty((128, NCH, BLOCK), dtype=np.float32)
        for b in range(BPC):
            s = s0 + b * BLOCK
            blk = blocks[s : s + BLOCK, s : s + BLOCK] * mask[s : s + BLOCK, s : s + BLOCK]
            for k in range(KCH):
                bk[:, b * KCH + k, :] = blk[k * 128 : (k + 1) * 128, :]
        bk = bk.reshape(128, NCH * BLOCK)
        in0 = np.concatenate([xt, bk[:, : 2 * KCH * BLOCK]], axis=1)
        in_maps.append(
            {
                "in0": np.ascontiguousarray(in0).astype(bf16),
                "in1": np.ascontiguousarray(bk[:, 2 * KCH * BLOCK :]).astype(bf16),
            }
        )
    return in_maps


def _run(x, blocks, mask, trace=False):
    from concourse import bass_utils

    _ensure_axon_ntff_hook()
    nc = _build_nc()
    in_maps = _prep_in_maps(x, blocks, mask)
    res = bass_utils.run_bass_kernel_spmd(
        nc, in_maps, core_ids=list(range(N_CORES)), trace=trace
    )
    out = np.empty((BATCH, N), dtype=np.float32)
    for d in range(N_CORES):
        out[:, d * COLS : (d + 1) * COLS] = res.results[d]["y"]
    return out, res


def kernel(x, blocks, mask):
    out, _ = _run(x, blocks, mask, trace=False)
    return out


# revision 6
# speedup vs baseline: 1.3691x; 1.0508x over previous
"""Block-diagonal matmul kernel for Trainium2 (8 NeuronCores, SPMD).

Reference computation: out = x @ (blocks * mask) with
  x      [64, 8192]  f32
  blocks [8192, 8192] f32
  mask   [8192, 8192] bool, block-diagonal (32 blocks of 256x256)

Only the 32 diagonal 256x256 blocks of `blocks` survive the mask, so the
real work is 32 independent [64,256] @ [256,256] matmuls.  Sharding
(per the expert/tensor-parallel hint): core d owns blocks 4d..4d+3 and
produces out[:, d*1024:(d+1)*1024].  x is sliced per-core (each block
only reads the matching 256 columns of x), outputs are concatenated on
the host - no cross-device communication needed.

Device-side layout (host prepares everything so every DMA is a plain
contiguous copy; inputs are pre-converted to bf16 on the host, which
halves HBM traffic and gives single-pass matmuls; accumulation stays
fp32 in PSUM):
  in0 [128, 1536] bf16 - x-slice^T (8 chunks of [128,64]) + blocks 0,1
  in1 [128, 1024] bf16 - blocks 2,3 (each block = 2 K-chunks of [128,256])
  y   [64, 1024]  f32  - output slice
Chunk c = 2b+k is K-half k of block b; matmul accumulates the 2 halves
of each block into one PSUM tile: y_b = sum_k xT_c.T @ B_c.
"""

import numpy as np

N_BLOCKS = 32
BLOCK = 256
N = N_BLOCKS * BLOCK  # 8192
BATCH = 64
N_CORES = 8
BPC = N_BLOCKS // N_CORES  # blocks per core = 4
COLS = BPC * BLOCK  # output columns per core = 1024
KCH = BLOCK // 128  # K-chunks per block = 2
NCH = BPC * KCH  # chunks per core = 8
XT_COLS = NCH * BATCH  # 512

_cached_nc = None


def _ensure_axon_ntff_hook():
    """The image's `antenv` package lacks `axon_hooks`, which
    run_bass_kernel_spmd imports unconditionally when tracing under axon.
    Inject a minimal shim and register the ctypes-based NTFF hook."""
    import sys
    import types

    try:
        import antenv.axon_hooks  # noqa: F401

        return
    except ImportError:
        pass
    try:
        import antenv
    except ImportError:
        return
    mod = types.ModuleType("antenv.axon_hooks")
    holder = {"h": None}
    mod.set_axon_ntff_profile_hook = lambda h: holder.__setitem__("h", h)
    mod.get_axon_ntff_profile_hook = lambda: holder["h"]
    sys.modules["antenv.axon_hooks"] = mod
    antenv.axon_hooks = mod
    try:
        from trn_agent_boot.trn_boot import _ntff_profile_via_ctypes

        h = _ntff_profile_via_ctypes("/opt/axon/libaxon_pjrt.so")
        if h is not None:
            mod.set_axon_ntff_profile_hook(h)
    except Exception:
        pass


def _build_nc():
    global _cached_nc
    if _cached_nc is not None:
        return _cached_nc

    import concourse.bacc as bacc
    import concourse.mybir as mybir
    import concourse.tile as tile
    import concourse.bass as bass

    f32 = mybir.dt.float32
    bf16 = mybir.dt.bfloat16
    nc = bacc.Bacc("TRN2", debug=False, num_devices=N_CORES)

    # in0 = xT (512 cols) + blocks 0,1 (1024 cols); in1 = blocks 2,3
    in0 = nc.dram_tensor("in0", [128, XT_COLS + 2 * KCH * BLOCK], bf16,
                         kind="ExternalInput")
    in1 = nc.dram_tensor("in1", [128, 2 * KCH * BLOCK], bf16,
                         kind="ExternalInput")
    # packed output: rows 0:64 = even blocks' batch, 64:128 = odd blocks';
    # cols g*256.. hold blocks {2g, 2g+1}
    y = nc.dram_tensor("y", [128, BPC // 2 * BLOCK], f32, kind="ExternalOutput")

    with tile.TileContext(nc) as tc:
        with (
            tc.tile_pool(name="sb", bufs=1) as pool,
            tc.tile_pool(name="ps", bufs=2, space=bass.MemorySpace.PSUM) as pp,
        ):
            t0 = pool.tile([128, XT_COLS + 2 * KCH * BLOCK], bf16)
            t1 = pool.tile([128, 2 * KCH * BLOCK], bf16)
            nc.sync.dma_start(t0[:], in0.ap())
            nc.scalar.dma_start(t1[:], in1.ap())
            out_sb = pool.tile([128, BPC // 2 * BLOCK], f32)

            xt = t0[:, 0:XT_COLS]

            def b_chunk(b, k):
                # [128, 256] slice for K-half k of block b
                c = (b % 2) * KCH + k
                t = t0 if b < 2 else t1
                off = (XT_COLS if b < 2 else 0) + c * BLOCK
                return t[:, off : off + BLOCK]

            for g in range(BPC // 2):  # group g = blocks {2g, 2g+1}
                acc = pp.tile([128, BLOCK], f32)
                for k in range(KCH):
                    for j in range(2):  # j=0 -> psum rows 0:64, j=1 -> 64:128
                        b = 2 * g + j
                        c = b * KCH + k
                        nc.tensor.matmul(
                            acc[64 * j : 64 * (j + 1), :],
                            xt[:, c * BATCH : (c + 1) * BATCH],
                            b_chunk(b, k),
                            start=(k == 0),
                            stop=(k == KCH - 1),
                            tile_position=(0, 64 * j),
                        )
                nc.vector.tensor_copy(
                    out_sb[:, g * BLOCK : (g + 1) * BLOCK], acc[:]
                )

            nc.scalar.dma_start(y.ap(), out_sb[:])

    nc.compile()
    _cached_nc = nc
    return nc


def _prep_in_maps(x, blocks, mask):
    import ml_dtypes

    bf16 = ml_dtypes.bfloat16
    x = np.ascontiguousarray(x, dtype=np.float32)
    in_maps = []
    for d in range(N_CORES):
        s0 = d * COLS
        # x slice transposed: [1024, 64] -> 8 chunks of [128, 64] -> [128, 512]
        xs = x[:, s0 : s0 + COLS].T.reshape(NCH, 128, BATCH)
        xt = np.ascontiguousarray(xs.transpose(1, 0, 2)).reshape(128, XT_COLS)
        # diagonal blocks (mask applied), K-chunked to [128, 256] slabs
        bk = np.empty((128, NCH, BLOCK), dtype=np.float32)
        for b in range(BPC):
            s = s0 + b * BLOCK
            blk = blocks[s : s + BLOCK, s : s + BLOCK] * mask[s : s + BLOCK, s : s + BLOCK]
            for k in range(KCH):
                bk[:, b * KCH + k, :] = blk[k * 128 : (k + 1) * 128, :]
        bk = bk.reshape(128, NCH * BLOCK)
        in0 = np.concatenate([xt, bk[:, : 2 * KCH * BLOCK]], axis=1)
        in_maps.append(
            {
                "in0": np.ascontiguousarray(in0).astype(bf16),
                "in1": np.ascontiguousarray(bk[:, 2 * KCH * BLOCK :]).astype(bf16),
            }
        )
    return in_maps


def _run(x, blocks, mask, trace=False):
    from concourse import bass_utils

    _ensure_axon_ntff_hook()
    nc = _build_nc()
    in_maps = _prep_in_maps(x, blocks, mask)
    res = bass_utils.run_bass_kernel_spmd(
        nc, in_maps, core_ids=list(range(N_CORES)), trace=trace
    )
    out = np.empty((BATCH, N), dtype=np.float32)
    for d in range(N_CORES):
        yl = res.results[d]["y"]  # [128, 512] packed
        for b in range(BPC):
            j, g = b % 2, b // 2
            out[:, d * COLS + b * BLOCK : d * COLS + (b + 1) * BLOCK] = yl[
                64 * j : 64 * (j + 1), g * BLOCK : (g + 1) * BLOCK
            ]
    return out, res


def kernel(x, blocks, mask):
    out, _ = _run(x, blocks, mask, trace=False)
    return out


# revision 7
# speedup vs baseline: 1.3841x; 1.0110x over previous
"""Block-diagonal matmul kernel for Trainium2 (8 NeuronCores, SPMD).

Reference computation: out = x @ (blocks * mask) with
  x      [64, 8192]  f32
  blocks [8192, 8192] f32
  mask   [8192, 8192] bool, block-diagonal (32 blocks of 256x256)

Only the 32 diagonal 256x256 blocks of `blocks` survive the mask, so the
real work is 32 independent [64,256] @ [256,256] matmuls.  Sharding
(per the expert/tensor-parallel hint): core d owns blocks 4d..4d+3 and
produces out[:, d*1024:(d+1)*1024].  x is sliced per-core (each block
only reads the matching 256 columns of x), outputs are concatenated on
the host - no cross-device communication needed.

Device-side layout (host prepares everything so every DMA is a plain
contiguous copy; inputs are pre-converted to bf16 on the host, which
halves HBM traffic and gives single-pass matmuls; accumulation stays
fp32 in PSUM):
  in0 [128, 1536] bf16 - x-slice^T (8 chunks of [128,64]) + blocks 0,1
  in1 [128, 1024] bf16 - blocks 2,3 (each block = 2 K-chunks of [128,256])
  y   [64, 1024]  f32  - output slice
Chunk c = 2b+k is K-half k of block b; matmul accumulates the 2 halves
of each block into one PSUM tile: y_b = sum_k xT_c.T @ B_c.
"""

import numpy as np

N_BLOCKS = 32
BLOCK = 256
N = N_BLOCKS * BLOCK  # 8192
BATCH = 64
N_CORES = 8
BPC = N_BLOCKS // N_CORES  # blocks per core = 4
COLS = BPC * BLOCK  # output columns per core = 1024
KCH = BLOCK // 128  # K-chunks per block = 2
NCH = BPC * KCH  # chunks per core = 8
XT_COLS = NCH * BATCH  # 512

_cached_nc = None


def _ensure_axon_ntff_hook():
    """The image's `antenv` package lacks `axon_hooks`, which
    run_bass_kernel_spmd imports unconditionally when tracing under axon.
    Inject a minimal shim and register the ctypes-based NTFF hook."""
    import sys
    import types

    try:
        import antenv.axon_hooks  # noqa: F401

        return
    except ImportError:
        pass
    try:
        import antenv
    except ImportError:
        return
    mod = types.ModuleType("antenv.axon_hooks")
    holder = {"h": None}
    mod.set_axon_ntff_profile_hook = lambda h: holder.__setitem__("h", h)
    mod.get_axon_ntff_profile_hook = lambda: holder["h"]
    sys.modules["antenv.axon_hooks"] = mod
    antenv.axon_hooks = mod
    try:
        from trn_agent_boot.trn_boot import _ntff_profile_via_ctypes

        h = _ntff_profile_via_ctypes("/opt/axon/libaxon_pjrt.so")
        if h is not None:
            mod.set_axon_ntff_profile_hook(h)
    except Exception:
        pass


def _build_nc():
    global _cached_nc
    if _cached_nc is not None:
        return _cached_nc

    import concourse.bacc as bacc
    import concourse.mybir as mybir
    import concourse.tile as tile
    import concourse.bass as bass

    f32 = mybir.dt.float32
    bf16 = mybir.dt.bfloat16
    nc = bacc.Bacc("TRN2", debug=False, num_devices=N_CORES)

    # in0 = xT (512 cols) + blocks 0,1 (1024 cols); in1 = blocks 2,3
    in0 = nc.dram_tensor("in0", [128, XT_COLS + 2 * KCH * BLOCK], bf16,
                         kind="ExternalInput")
    in1 = nc.dram_tensor("in1", [128, 2 * KCH * BLOCK], bf16,
                         kind="ExternalInput")
    # packed output: rows 0:64 = even blocks' batch, 64:128 = odd blocks';
    # cols g*256.. hold blocks {2g, 2g+1}
    y = nc.dram_tensor("y", [128, BPC // 2 * BLOCK], f32, kind="ExternalOutput")

    with tile.TileContext(nc) as tc:
        with (
            tc.tile_pool(name="sb", bufs=1) as pool,
            tc.tile_pool(name="ps", bufs=2, space=bass.MemorySpace.PSUM) as pp,
        ):
            # separate SBUF tiles per DMA so each block's matmuls only wait
            # for their own transfer; 2 DMAs per HWDGE ring (SP + ACT)
            t_xb0 = pool.tile([128, XT_COLS + KCH * BLOCK], bf16, name="t_xb0")
            t_b1 = pool.tile([128, KCH * BLOCK], bf16, name="t_b1")
            t_b2 = pool.tile([128, KCH * BLOCK], bf16, name="t_b2")
            t_b3 = pool.tile([128, KCH * BLOCK], bf16, name="t_b3")
            nc.sync.dma_start(t_xb0[:], in0.ap()[:, 0 : XT_COLS + KCH * BLOCK])
            nc.scalar.dma_start(t_b1[:], in0.ap()[:, XT_COLS + KCH * BLOCK :])
            nc.sync.dma_start(t_b2[:], in1.ap()[:, 0 : KCH * BLOCK])
            nc.scalar.dma_start(t_b3[:], in1.ap()[:, KCH * BLOCK :])

            xt = t_xb0[:, 0:XT_COLS]
            bt = {
                0: t_xb0[:, XT_COLS:],
                1: t_b1[:],
                2: t_b2[:],
                3: t_b3[:],
            }
            outs = []
            for g in range(BPC // 2):  # group g = blocks {2g, 2g+1}
                acc = pp.tile([128, BLOCK], f32)
                for k in range(KCH):
                    for j in range(2):  # j=0 -> psum rows 0:64, j=1 -> 64:128
                        b = 2 * g + j
                        c = b * KCH + k
                        nc.tensor.matmul(
                            acc[64 * j : 64 * (j + 1), :],
                            xt[:, c * BATCH : (c + 1) * BATCH],
                            bt[b][:, k * BLOCK : (k + 1) * BLOCK],
                            start=(k == 0),
                            stop=(k == KCH - 1),
                            tile_position=(0, 64 * j),
                        )
                o = pool.tile([128, BLOCK], f32, name=f"out{g}")
                nc.vector.tensor_copy(o[:], acc[:])
                outs.append(o)

            nc.scalar.dma_start(y.ap()[:, 0:BLOCK], outs[0][:])
            nc.sync.dma_start(y.ap()[:, BLOCK:], outs[1][:])

    nc.compile()
    _cached_nc = nc
    return nc


def _prep_in_maps(x, blocks, mask):
    import ml_dtypes

    bf16 = ml_dtypes.bfloat16
    x = np.ascontiguousarray(x, dtype=np.float32)
    in_maps = []
    for d in range(N_CORES):
        s0 = d * COLS
        # x slice transposed: [1024, 64] -> 8 chunks of [128, 64] -> [128, 512]
        xs = x[:, s0 : s0 + COLS].T.reshape(NCH, 128, BATCH)
        xt = np.ascontiguousarray(xs.transpose(1, 0, 2)).reshape(128, XT_COLS)
        # diagonal blocks (mask applied), K-chunked to [128, 256] slabs
        bk = np.empty((128, NCH, BLOCK), dtype=np.float32)
        for b in range(BPC):
            s = s0 + b * BLOCK
            blk = blocks[s : s + BLOCK, s : s + BLOCK] * mask[s : s + BLOCK, s : s + BLOCK]
            for k in range(KCH):
                bk[:, b * KCH + k, :] = blk[k * 128 : (k + 1) * 128, :]
        bk = bk.reshape(128, NCH * BLOCK)
        in0 = np.concatenate([xt, bk[:, : 2 * KCH * BLOCK]], axis=1)
        in_maps.append(
            {
                "in0": np.ascontiguousarray(in0).astype(bf16),
                "in1": np.ascontiguousarray(bk[:, 2 * KCH * BLOCK :]).astype(bf16),
            }
        )
    return in_maps


def _run(x, blocks, mask, trace=False):
    from concourse import bass_utils

    _ensure_axon_ntff_hook()
    nc = _build_nc()
    in_maps = _prep_in_maps(x, blocks, mask)
    res = bass_utils.run_bass_kernel_spmd(
        nc, in_maps, core_ids=list(range(N_CORES)), trace=trace
    )
    out = np.empty((BATCH, N), dtype=np.float32)
    for d in range(N_CORES):
        yl = res.results[d]["y"]  # [128, 512] packed
        for b in range(BPC):
            j, g = b % 2, b // 2
            out[:, d * COLS + b * BLOCK : d * COLS + (b + 1) * BLOCK] = yl[
                64 * j : 64 * (j + 1), g * BLOCK : (g + 1) * BLOCK
            ]
    return out, res


def kernel(x, blocks, mask):
    out, _ = _run(x, blocks, mask, trace=False)
    return out
